# revision 1
# baseline (speedup 1.0000x reference)
"""CenterLoss kernel for Trainium2 (8 NeuronCores, data-parallel over N).

loss = sum_{n,c,w} act[n,c,w] * dist[n,c,w],  clipped at 1e-6, where
  dist[n,c,w] = ||x[n,:,w] - ctr[:,c]||^2 = x2[n,w] - 2*xc[n,c,w] + c2[c]

Per-core strategy (2 of 16 n-values per core):
  - x is cast to bf16 during the DMA (SWDGE) and squared on ScalarE into the
    other half of a [128, CH] tile, giving rhs = [x ; x^2] stacked along the
    contraction dim.  For the second n the layout is flipped ([x^2 ; x], DMA
    into partitions 64:128) so x-DMAs cover both SBUF port halves
    (partitions 0:64 only reach the 8 even ports).
  - One K=128 bf16 matmul per 512-column PSUM bank computes
    dist' = -2*xc + x2 directly (lhsT = [-2c ; 1], mirrored for the flip).
    bf16 matters: fp32 matmuls run at half column rate on the PE.
  - One fused DVE scalar_tensor_tensor per [80,1024] PSUM tile computes
    (dist' + c2) * act and its per-partition row-sum into a column of a
    [80, 32] accumulator (c2 enters in fp32 here; no extra pass).
  - Tail: reduce the accumulator, then ones^T @ racc -> [1,1] on PE.
    Host sums the 8 per-core partials and applies the clip.
"""

import os
import sys

import numpy as np

for _p in ("/opt/trn_rl_repo",):
    if _p not in sys.path and os.path.isdir(_p):
        sys.path.insert(0, _p)

N, D, C, W = 16, 64, 80, 16384
NCORES = 8
NPER = N // NCORES  # 2
CH = 2048  # w-chunk per DMA
NCH = W // CH
SUB = 1024  # free-dim per PSUM tile / fused DVE op
MMN = 512  # matmul free dim (one PSUM bank)
NACC = NPER * NCH * (CH // SUB)  # 32 accumulator columns

_CACHE = {}


def _build_bass():
    import concourse.bacc as bacc
    import concourse.tile as tile
    from concourse import mybir

    fp32 = mybir.dt.float32
    bf16 = mybir.dt.bfloat16
    Alu = mybir.AluOpType

    nc = bacc.Bacc("TRN2", target_bir_lowering=False, num_swdge_queues=4)

    xs = nc.dram_tensor("xs", [NPER * D, W], fp32, kind="ExternalInput")
    acts = nc.dram_tensor("acts", [NPER * C, W], fp32, kind="ExternalInput")
    wA = nc.dram_tensor("wA", [128, C], bf16, kind="ExternalInput")  # [-2c; 1]
    wB = nc.dram_tensor("wB", [128, C], bf16, kind="ExternalInput")  # [1; -2c]
    wtc = nc.dram_tensor("wtc", [C, 2], fp32, kind="ExternalInput")  # [c2 | 1]
    out = nc.dram_tensor("out", [1, 1], fp32, kind="ExternalOutput")

    from contextlib import ExitStack

    with tile.TileContext(nc) as tc, ExitStack() as ctx:
        consts = ctx.enter_context(tc.tile_pool(name="consts", bufs=1))
        xpool = ctx.enter_context(tc.tile_pool(name="xpool", bufs=4))
        apool = ctx.enter_context(tc.tile_pool(name="apool", bufs=4))
        spool = ctx.enter_context(tc.tile_pool(name="spool", bufs=2))
        rpool = ctx.enter_context(tc.tile_pool(name="rpool", bufs=1))
        opool = ctx.enter_context(tc.tile_pool(name="opool", bufs=1))
        pdist = ctx.enter_context(tc.tile_pool(name="pdist", bufs=3, space="PSUM"))
        psmall = ctx.enter_context(tc.tile_pool(name="psmall", bufs=1, space="PSUM"))

        wA_t = consts.tile([128, C], bf16)
        nc.sync.dma_start(out=wA_t[:], in_=wA[:, :])
        wB_t = consts.tile([128, C], bf16)
        nc.sync.dma_start(out=wB_t[:], in_=wB[:, :])
        wtc_t = consts.tile([C, 2], fp32)
        nc.sync.dma_start(out=wtc_t[:], in_=wtc[:, :])

        pfin = psmall.tile([1, 1], fp32)
        racc_all = rpool.tile([C, NACC], fp32)

        iacc = 0
        for ci in range(NCH):
            w0 = ci * CH
            for ni in range(NPER):
                flip = ni == 1
                w_t = wB_t if flip else wA_t
                xx = xpool.tile([128, CH], bf16, tag="xx")
                # First iteration: DMA+square in 512-col slices so the first
                # matmul/fused-op can start ~4x sooner (pipeline fill).
                nslc = 4 if (ci == 0 and ni == 0) else 1
                for j in range(nslc):
                    lo, hi = w0 + j * (CH // nslc), w0 + (j + 1) * (CH // nslc)
                    s0, s1 = j * (CH // nslc), (j + 1) * (CH // nslc)
                    if not flip:
                        nc.gpsimd.dma_start(out=xx[0:64, s0:s1], in_=xs[0:D, lo:hi])
                        nc.scalar.square(out=xx[64:128, s0:s1], in_=xx[0:64, s0:s1])
                    else:
                        nc.gpsimd.dma_start(
                            out=xx[64:128, s0:s1], in_=xs[D : 2 * D, lo:hi]
                        )
                        nc.scalar.square(out=xx[0:64, s0:s1], in_=xx[64:128, s0:s1])
                at = apool.tile([C, CH], fp32, tag="at")
                nc.sync.dma_start(
                    out=at[:], in_=acts[ni * C : (ni + 1) * C, w0 : w0 + CH]
                )

                for si in range(CH // SUB):
                    pd = pdist.tile([C, SUB], fp32, tag="pd")
                    for mi in range(SUB // MMN):
                        s = si * SUB + mi * MMN
                        nc.tensor.matmul(
                            pd[:, mi * MMN : (mi + 1) * MMN],
                            w_t[:],
                            xx[:, s : s + MMN],
                            start=True,
                            stop=True,
                        )
                    scr = spool.tile([C, SUB], fp32, tag="scr")
                    nc.vector.scalar_tensor_tensor(
                        out=scr[:],
                        in0=pd[:],
                        scalar=wtc_t[:, 0:1],
                        in1=at[:, si * SUB : (si + 1) * SUB],
                        op0=Alu.add,
                        op1=Alu.mult,
                        accum_out=racc_all[:, iacc : iacc + 1],
                    )
                    iacc += 1

        # loss_core = ones^T @ (row-sums of racc_all)
        racc = opool.tile([C, 1], fp32, tag="racc")
        nc.vector.tensor_reduce(
            out=racc[:], in_=racc_all[:], axis=mybir.AxisListType.X, op=Alu.add
        )
        nc.tensor.matmul(pfin[:], wtc_t[:, 1:2], racc[:], start=True, stop=True)
        osb = opool.tile([1, 1], fp32, tag="osb")
        nc.vector.tensor_copy(osb[:], pfin[:])
        nc.sync.dma_start(out=out[:, :], in_=osb[:])

    nc.compile()
    return nc


def _get_nc():
    if "nc" not in _CACHE:
        _CACHE["nc"] = _build_bass()
    return _CACHE["nc"]


def kernel(x, c, act):
    import ml_dtypes
    from concourse.bass_utils import run_bass_kernel_spmd

    x = np.ascontiguousarray(np.asarray(x), dtype=np.float32)
    c = np.ascontiguousarray(np.asarray(c), dtype=np.float32)
    act = np.ascontiguousarray(np.asarray(act), dtype=np.float32)
    assert x.shape == (N, D, W) and c.shape == (D, C) and act.shape == (N, C, W)

    bf16 = ml_dtypes.bfloat16
    c2 = np.sum(c * c, axis=0, dtype=np.float32)  # [C]
    ones_dc = np.ones((D, C), dtype=np.float32)
    wA = np.ascontiguousarray(
        np.concatenate([-2.0 * c, ones_dc], axis=0), dtype=bf16
    )  # [128, C]
    wB = np.ascontiguousarray(
        np.concatenate([ones_dc, -2.0 * c], axis=0), dtype=bf16
    )  # [128, C]
    wtc = np.ascontiguousarray(
        np.stack([c2, np.ones(C, dtype=np.float32)], axis=1), dtype=np.float32
    )

    in_maps = []
    for k in range(NCORES):
        in_maps.append(
            {
                "xs": np.ascontiguousarray(x[NPER * k : NPER * (k + 1)]).reshape(NPER * D, W),
                "acts": np.ascontiguousarray(act[NPER * k : NPER * (k + 1)]).reshape(NPER * C, W),
                "wA": wA,
                "wB": wB,
                "wtc": wtc,
            }
        )

    res = run_bass_kernel_spmd(_get_nc(), in_maps, core_ids=list(range(NCORES)))
    total = np.float32(0.0)
    for r in res.results:
        total = np.float32(total + np.float32(r["out"][0, 0]))
    return np.maximum(np.float32(total), np.float32(1e-6))

